# revision 1
# baseline (speedup 1.0000x reference)
"""Bass/Trainium2 kernel for nn_DKPF_Attention (dense_cnn).

Sharding: 8 cores = (batch b in 0..3) x (H-half in 0..1). Each core computes
the full pipeline for its 32-row slice; the only cross-core data is one tiny
AllReduce carrying BatchNorm batch stats (sum/sumsq of the depth-encoder conv)
and the global-average-pool partial sums.

Self-contained: shapes/weight transforms hardcoded; no external file reads.
"""

import os

import numpy as np

import concourse.bass as bass
import concourse.bacc as bacc
import concourse.tile as tile
from concourse import mybir
from concourse.bass_utils import run_bass_kernel_spmd

F32 = mybir.dt.float32
F32R = mybir.dt.float32r
AT = mybir.ActivationFunctionType
OP = mybir.AluOpType
AX = mybir.AxisListType

B, C, D, H, W = 4, 128, 128, 64, 64
D4 = D // 4            # 32
K2 = 83                # 9 + 25 + 49
K2P = 96               # padded to 3 x 32 for selector matmuls
A = 3
N_CORES = 8
RO = 32                # output rows per core
NQ = 4                 # 512-px chunks
QR = RO // NQ          # 8 rows per chunk
EPS = 1e-5
NTOT = float(B * H * W)  # BN batch-stat count

# branch table: (k, pad, j-offset into the 83 kernel rows)
BR = ((3, 1, 0), (5, 2, 9), (7, 3, 34))

RGB_R, RGB_C = RO + 12, W + 12    # 44 x 76 (halo 6 = dw 3 + dyn 3)
DEP_R, DEP_C = RO + 2, W + 2      # 34 x 66 (halo 1)


def _bcast_free(ap2, rows, cols):
    """[P,1] slice -> [P,rows,cols] free-dim broadcast AP."""
    return bass.AP(tensor=ap2.tensor, offset=ap2.offset,
                   ap=[ap2.ap[0], [0, rows], [0, cols]])


def _build():
    ABL = set(os.environ.get("KERNEL_ABLATE", "").split(","))
    nc = bacc.Bacc()
    dp = nc.declare_dram_parameter

    rgb = dp("rgb", [C, RGB_R, RGB_C], F32, isOutput=False)
    dep = dp("dep", [D, DEP_R, DEP_C], F32R, isOutput=False)
    dg_w = dp("dg_w", [D, 9, D4 + A], F32R, isOutput=False)     # depth-enc + gate depth-half lhsT
    gr_w = dp("gr_w", [C, 9, A], F32R, isOutput=False)          # gate rgb-half lhsT
    res_wt = dp("res_wt", [C, C], F32R, isOutput=False)
    kg_wt = dp("kg_wt", [D4, K2], F32R, isOutput=False)
    dw_w = dp("dw_w", [C, K2], F32, isOutput=False)             # depthwise taps
    b357 = dp("b357", [C, 3], F32, isOutput=False)              # depthwise biases
    ebr = dp("ebr", [A, K2], F32R, isOutput=False)              # branch one-hot selector
    on3 = dp("on3", [A, K2], F32R, isOutput=False)              # ones
    one1 = dp("one1", [1, C], F32R, isOutput=False)             # ones row
    sel32 = dp("sel32", [96, 32, C], F32R, isOutput=False)      # per 32-group: one-hot row selectors
    a1w = dp("a1w", [C, 2, A], F32, isOutput=False)             # att 1x1 #1 (pre-divided by H*W)
    a2w = dp("a2w", [A, A], F32, isOutput=False)
    one1f = dp("one1f", [1, C], F32, isOutput=False)
    on3f = dp("on3f", [A, 1], F32, isOutput=False)
    a1b = dp("a1b", [A, 1], F32, isOutput=False)
    a2b = dp("a2b", [A, 1], F32, isOutput=False)
    bn_g = dp("bn_g", [D4, 1], F32, isOutput=False)
    bn_b = dp("bn_b", [D4, 1], F32, isOutput=False)
    kg_b = dp("kg_b", [K2, 1], F32, isOutput=False)
    gate_b = dp("gate_b", [A, 1], F32, isOutput=False)
    res_b = dp("res_b", [C, 1], F32, isOutput=False)
    gmask = dp("gmask", [C, 8], F32, isOutput=False)            # sample-slot one-hot
    zpad = dp("zpad", [K2P - K2, RO, W], F32R, isOutput=False)  # zeros for kern_f padding
    half_f = dp("half_f", [C, 2], F32, isOutput=False)          # halo-row keep masks

    out = dp("out", [C, RO, W], F32, isOutput=True)

    with tile.TileContext(nc) as tc:
        with (
            nc.allow_low_precision(reason="f32r matmul rounding is intentional"),
            tc.tile_pool(name="cw", bufs=1) as cw,      # constants/weights
            tc.tile_pool(name="big", bufs=1) as big,    # persistent activations
            tc.tile_pool(name="ev", bufs=1) as ev,      # small sbuf
            tc.tile_pool(name="pq", bufs=1, space="PSUM") as pq,
            tc.tile_pool(name="dram", bufs=1, space="DRAM") as dram,
        ):
            # ---------------- loads ----------------
            rgb_t = big.tile([C, RGB_R, RGB_C], F32)
            dep_t = big.tile([D, DEP_R, DEP_C], F32R)
            nc.sync.dma_start(out=rgb_t, in_=rgb[...])
            nc.sync.dma_start(out=dep_t, in_=dep[...])

            dg_wt = cw.tile([D, 9, D4 + A], F32R)
            gr_wt = cw.tile([C, 9, A], F32R)
            res_wtt = cw.tile([C, C], F32R)
            kg_wtt = cw.tile([D4, K2], F32R)
            dw_wt = cw.tile([C, K2], F32)
            b357t = cw.tile([C, 3], F32)
            ebr_t = cw.tile([A, K2], F32R)
            on3_t = cw.tile([A, K2], F32R)
            one1_t = cw.tile([1, C], F32R)
            sel32_t = cw.tile([96, 32, C], F32R)
            a1w_t = cw.tile([C, 2, A], F32)
            a2w_t = cw.tile([A, A], F32)
            one1f_t = cw.tile([1, C], F32)
            on3f_t = cw.tile([A, 1], F32)
            a1b_t = cw.tile([A, 1], F32)
            a2b_t = cw.tile([A, 1], F32)
            bn_g_t = cw.tile([D4, 1], F32)
            bn_b_t = cw.tile([D4, 1], F32)
            kg_b_t = cw.tile([K2, 1], F32)
            gate_b_t = cw.tile([A, 1], F32)
            res_b_t = cw.tile([C, 1], F32)
            gmask_t = cw.tile([C, 8], F32)
            half_t = cw.tile([C, 2], F32)
            for dst, src in (
                (dg_wt, dg_w), (gr_wt, gr_w), (res_wtt, res_wt), (kg_wtt, kg_wt),
                (dw_wt, dw_w), (b357t, b357), (ebr_t, ebr), (on3_t, on3),
                (one1_t, one1), (sel32_t, sel32), (a1w_t, a1w), (a2w_t, a2w),
                (a1b_t, a1b), (a2b_t, a2b), (one1f_t, one1f), (on3f_t, on3f), (bn_g_t, bn_g), (bn_b_t, bn_b),
                (kg_b_t, kg_b), (gate_b_t, gate_b), (res_b_t, res_b),
                (gmask_t, gmask), (half_t, half_f),
            ):
                nc.sync.dma_start(out=dst, in_=src[...])

            # f32r copy of rgb for matmul paths (ACT rounds)
            rgb_r = big.tile([C, RGB_R, RGB_C], F32R)
            nc.scalar.activation(out=rgb_r, in_=rgb_t, func=AT.Copy)

            # ---------------- depth-encoder conv (+ gate depth-half) ----------------
            e_sb = big.tile([D4, RO, W], F32)
            g_dep = big.tile([A, RO, W], F32)
            s1p = big.tile([D4, NQ], F32)
            s2p = big.tile([D4, NQ], F32)
            e_scr = big.tile([D4, RO, W], F32)  # square scratch
            for q in range(NQ):
                e_ps = pq.tile([D4 + A, QR, W], F32, tag="ps1", bufs=4)
                for j in (range(1) if "mats" in ABL else range(9)):
                    dy, dx = j // 3, j % 3
                    nc.tensor.matmul(
                        e_ps,
                        lhsT=dg_wt[:, j, :],
                        rhs=dep_t[:, dy + QR * q: dy + QR * q + QR, dx: dx + W],
                        start=(j == 0), stop=(j == 8),
                    )
                qs = slice(QR * q, QR * q + QR)
                nc.vector.tensor_reduce(out=s1p[:, q: q + 1], in_=e_ps[:D4], axis=AX.XY, op=OP.add)
                nc.scalar.activation(out=e_scr[:, qs], in_=e_ps[:D4], func=AT.Square,
                                     accum_out=s2p[:, q: q + 1])
                nc.scalar.activation(out=e_sb[:, qs], in_=e_ps[:D4], func=AT.Copy)
                nc.scalar.activation(out=g_dep[:, qs], in_=e_ps[D4:], func=AT.Copy)

            # ---------------- GAP partials + stats -> AllReduce ----------------
            cc_sb = ev.tile([C, 10], F32)
            nc.vector.memset(cc_sb, 0.0)
            sums2 = ev.tile([C, 2], F32)
            nc.vector.tensor_reduce(out=sums2[:, 0:1], in_=rgb_t[:, 6:6 + RO, 6:6 + W], axis=AX.XY, op=OP.add)
            nc.vector.tensor_reduce(out=sums2[:, 1:2], in_=dep_t[:, 1:1 + RO, 1:1 + W].bitcast(F32),
                                    axis=AX.XY, op=OP.add)
            s2b = bass.AP(tensor=sums2.tensor, offset=sums2.offset,
                          ap=[sums2.ap[0], sums2.ap[1], [0, 4]])
            nc.vector.tensor_tensor(out=cc_sb[:, 0:8].rearrange("p (a b) -> p a b", a=2),
                                    in0=s2b, in1=gmask_t.rearrange("p (a b) -> p a b", a=2),
                                    op=OP.mult)
            nc.vector.tensor_reduce(out=cc_sb[:D4, 8:9], in_=s1p, axis=AX.X, op=OP.add)
            nc.vector.tensor_reduce(out=cc_sb[:D4, 9:10], in_=s2p, axis=AX.X, op=OP.add)

            cc_in = dram.tile([C, 10], F32)
            cc_out = dram.tile([C, 10], F32)
            nc.sync.dma_start(out=cc_in, in_=cc_sb)
            if "cc" in ABL:
                nc.sync.dma_start(out=cc_out[:, :], in_=cc_in[:, :])
            else:
                nc.gpsimd.collective_compute(
                    "AllReduce", OP.add,
                    replica_groups=[list(range(N_CORES))],
                    ins=[cc_in[:, :]], outs=[cc_out[:, :]],
                )
            cc_r = ev.tile([C, 10], F32)
            nc.sync.dma_start(out=cc_r, in_=cc_out[:, :])

            # ---------------- BN constants ----------------
            mstat = ev.tile([D4, 4], F32)  # cols: m, ex2, sd, 1/sd
            nc.scalar.activation(out=mstat[:, 0:1], in_=cc_r[:D4, 8:9], func=AT.Copy, scale=1.0 / NTOT)
            nc.scalar.activation(out=mstat[:, 1:2], in_=cc_r[:D4, 9:10], func=AT.Copy, scale=1.0 / NTOT)
            m2 = ev.tile([D4, 1], F32)
            nc.vector.tensor_tensor(out=m2, in0=mstat[:, 0:1], in1=mstat[:, 0:1], op=OP.mult)
            nc.vector.tensor_tensor(out=mstat[:, 2:3], in0=mstat[:, 1:2], in1=m2, op=OP.subtract)
            epsb = ev.tile([D4, 1], F32)
            nc.vector.memset(epsb, EPS)
            nc.scalar.activation(out=mstat[:, 2:3], in_=mstat[:, 2:3], func=AT.Sqrt, bias=epsb)
            nc.vector.reciprocal(out=mstat[:, 3:4], in_=mstat[:, 2:3])
            bnsc = ev.tile([D4, 1], F32)
            bnsh = ev.tile([D4, 1], F32)
            nc.vector.tensor_tensor(out=bnsc, in0=mstat[:, 3:4], in1=bn_g_t, op=OP.mult)
            msc = ev.tile([D4, 1], F32)
            nc.vector.tensor_tensor(out=msc, in0=mstat[:, 0:1], in1=bnsc, op=OP.mult)
            nc.vector.tensor_tensor(out=bnsh, in0=bn_b_t, in1=msc, op=OP.subtract)

            # depth_enc = relu(e*scale + shift), rounded to f32r for the kw matmul
            de_r = big.tile([D4, RO, W], F32R)
            nc.scalar.activation(out=de_r, in_=e_sb, func=AT.Relu, bias=bnsh, scale=bnsc)

            # ---------------- attention (tiny) ----------------
            apk = ev.tile([C, 2, 4], F32)
            nc.vector.tensor_tensor(out=apk, in0=cc_r[:, 0:8].rearrange("p (a b) -> p a b", a=2),
                                    in1=gmask_t.rearrange("p (a b) -> p a b", a=2), op=OP.mult)
            a_sel = ev.tile([C, 2], F32)
            nc.vector.tensor_reduce(out=a_sel, in_=apk, axis=AX.X, op=OP.add)
            a1_ps = pq.tile([A, 1], F32, tag="tiny", bufs=2)
            nc.tensor.matmul(a1_ps, lhsT=a1w_t[:, 0, :], rhs=a_sel[:, 0:1], start=True, stop=False)
            nc.tensor.matmul(a1_ps, lhsT=a1w_t[:, 1, :], rhs=a_sel[:, 1:2], start=False, stop=True)
            h_r = ev.tile([A, 1], F32)
            nc.scalar.activation(out=h_r, in_=a1_ps, func=AT.Relu, bias=a1b_t)
            a2_ps = pq.tile([A, 1], F32, tag="tiny", bufs=2)
            nc.tensor.matmul(a2_ps, lhsT=a2w_t, rhs=h_r, start=True, stop=True)
            exq = ev.tile([A, 1], F32)
            nc.scalar.activation(out=exq, in_=a2_ps, func=AT.Exp, bias=a2b_t)
            sq_ps = pq.tile([1, 1], F32, tag="tiny", bufs=2)
            nc.tensor.matmul(sq_ps, lhsT=exq, rhs=on3f_t, start=True, stop=True)
            rs = ev.tile([1, 1], F32)
            nc.vector.reciprocal(out=rs, in_=sq_ps)
            rb_ps = pq.tile([A, 1], F32, tag="tiny", bufs=2)
            nc.tensor.matmul(rb_ps, lhsT=one1f_t[0:1, 0:A], rhs=rs, start=True, stop=True)
            att = ev.tile([A, 1], F32)
            nc.vector.tensor_tensor(out=att, in0=exq, in1=rb_ps, op=OP.mult)

            # ---------------- gates ----------------
            exin = big.tile([A, RO, W], F32)
            for q in range(NQ):
                g_ps = pq.tile([A, QR, W], F32, tag="ps1", bufs=4)
                for j in (range(1) if "mats" in ABL else range(9)):
                    dy, dx = j // 3, j % 3
                    nc.tensor.matmul(
                        g_ps,
                        lhsT=gr_wt[:, j, :],
                        rhs=rgb_r[:, 5 + dy + QR * q: 5 + dy + QR * q + QR, 5 + dx: 5 + dx + W],
                        start=(j == 0), stop=(j == 8),
                    )
                qs = slice(QR * q, QR * q + QR)
                nc.vector.scalar_tensor_tensor(out=exin[:, qs], in0=g_ps, scalar=gate_b_t,
                                               in1=g_dep[:, qs], op0=OP.add, op1=OP.add)
            ex_r = big.tile([A, RO, W], F32R)     # exp(gate), for denominator
            nc.scalar.activation(out=ex_r, in_=exin, func=AT.Exp)
            exa_r = big.tile([A, RO, W], F32R)    # exp(gate)*att, for numerator
            nc.vector.tensor_scalar_mul(out=exa_r, in0=ex_r.bitcast(F32), scalar1=att)

            # ---------------- kern' assembly ----------------
            # kw = kg_wt.T @ depth_enc ; kern = (kw + kg_b) * bcast(exa) * bcast(1/sum ex)
            exab_sb = big.tile([K2, RO, W], F32)
            kern_t = big.tile([K2, RO, W], F32)
            kern_f = big.tile([K2P, RO, W], F32R)
            rsum_r = big.tile([1, RO, W], F32R)
            nc.sync.dma_start(out=kern_f[K2:K2P], in_=zpad[...])
            for q in range(NQ):
                qs = slice(QR * q, QR * q + QR)
                sum_ps = pq.tile([1, QR, W], F32, tag="ps1", bufs=4)
                nc.tensor.matmul(sum_ps, lhsT=on3_t[:, 0:1], rhs=ex_r[:, qs], start=True, stop=True)
                nc.vector.reciprocal(out=rsum_r[:, qs], in_=sum_ps)
                exab_ps = pq.tile([K2, QR, W], F32, tag="ps1", bufs=4)
                nc.tensor.matmul(exab_ps, lhsT=ebr_t, rhs=exa_r[:, qs], start=True, stop=True)
                nc.scalar.activation(out=exab_sb[:, qs], in_=exab_ps, func=AT.Copy)
                kw_ps = pq.tile([K2, QR, W], F32, tag="ps1", bufs=4)
                nc.tensor.matmul(kw_ps, lhsT=kg_wtt, rhs=de_r[:, qs], start=True, stop=True)
                nc.vector.scalar_tensor_tensor(out=kern_t[:, qs], in0=kw_ps, scalar=kg_b_t,
                                               in1=exab_sb[:, qs], op0=OP.add, op1=OP.mult)
                den_ps = pq.tile([K2, QR, W], F32, tag="ps1", bufs=4)
                nc.tensor.matmul(den_ps, lhsT=one1_t[0:1, 0:K2], rhs=rsum_r[:, qs], start=True, stop=True)
                nc.vector.tensor_tensor(out=kern_f[:K2, qs], in0=kern_t[:, qs], in1=den_ps, op=OP.mult)

            # ---------------- depthwise convs (DVE) ----------------
            f_ts = []
            for bi, (k, p, joff) in enumerate(BR):
                rows, cols = RO + 2 * p, W + 2 * p
                f_t = big.tile([C, rows, cols], F32, name=f"f{k}")
                f_ts.append(f_t)
                base = 6 - 2 * p
                bias_b = _bcast_free(b357t[:, bi:bi + 1], rows, cols)
                for j in (range(1) if "dw" in ABL else range(k * k)):
                    di, dj = j // k, j % k
                    src = rgb_t[:, base + di: base + di + rows, base + dj: base + dj + cols]
                    nc.vector.scalar_tensor_tensor(
                        out=f_t, in0=src, scalar=dw_wt[:, joff + j: joff + j + 1],
                        in1=(bias_b if j == 0 else f_t), op0=OP.mult, op1=OP.add)
                # zero outside-image halo
                nc.gpsimd.memset(f_t[:, :, 0:p], 0.0)
                nc.gpsimd.memset(f_t[:, :, W + p: W + 2 * p], 0.0)
                nc.vector.tensor_scalar_mul(out=f_t[:, 0:p, :], in0=f_t[:, 0:p, :],
                                            scalar1=half_t[:, 0:1])
                nc.vector.tensor_scalar_mul(out=f_t[:, RO + p: RO + 2 * p, :],
                                            in0=f_t[:, RO + p: RO + 2 * p, :],
                                            scalar1=half_t[:, 1:2])

            # ---------------- dynamic conv + res ----------------
            acc = big.tile([C, RO, W], F32)
            res_sb = big.tile([C, RO, W], F32)
            for q in range(NQ):
                qs = slice(QR * q, QR * q + QR)
                res_ps = pq.tile([C, QR, W], F32, tag="ps1", bufs=4)
                nc.tensor.matmul(res_ps, lhsT=res_wtt,
                                 rhs=rgb_r[:, 6 + QR * q: 6 + QR * q + QR, 6: 6 + W],
                                 start=True, stop=True)
                nc.scalar.activation(out=res_sb[:, qs], in_=res_ps, func=AT.Copy)
                first = True
                for bi, (k, p, joff) in enumerate(BR):
                    f_t = f_ts[bi]
                    jrange = range(0) if "dyn" in ABL else range(k * k)
                    if "dyn" in ABL and bi == 0:
                        nc.vector.tensor_tensor(out=acc[:, qs], in0=f_t[:, QR*q:QR*q+QR, 0:W],
                                                in1=f_t[:, QR*q:QR*q+QR, 0:W], op=OP.mult)
                    for j in jrange:
                        di, dj = j // k, j % k
                        jj = joff + j
                        gbase = (jj // 32) * 32
                        r = jj - gbase
                        kb_ps = pq.tile([C, QR, W], F32, tag="kb", bufs=2)
                        nc.tensor.matmul(kb_ps, lhsT=sel32_t[gbase: gbase + 32, r, :],
                                         rhs=kern_f[gbase: gbase + 32, qs],
                                         start=True, stop=True)
                        fsl = f_t[:, di + QR * q: di + QR * q + QR, dj: dj + W]
                        if first:
                            nc.vector.tensor_tensor(out=acc[:, qs], in0=fsl, in1=kb_ps, op=OP.mult)
                            first = False
                        else:
                            tmp = ev.tile([C, QR, W], F32, tag="tmp", bufs=4)
                            nc.vector.tensor_tensor(out=tmp, in0=fsl, in1=kb_ps, op=OP.mult)
                            eng = nc.vector if "gpadd" in ABL else nc.gpsimd
                            eng.tensor_tensor(out=acc[:, qs], in0=acc[:, qs], in1=tmp, op=OP.add)

            # ---------------- combine + store ----------------
            out_sb = big.tile([C, RO, W], F32)
            nc.vector.scalar_tensor_tensor(out=out_sb, in0=res_sb, scalar=res_b_t,
                                           in1=acc, op0=OP.add, op1=OP.add)
            nc.sync.dma_start(out=out[...], in_=out_sb)

    nc.finalize()
    return nc


_NC = None


def _get_nc():
    global _NC
    if _NC is None:
        _NC = _build()
    return _NC


def _prep_common(i):
    """Weight transforms shared by all cores."""
    de_w, kg_w = i["de_w"], i["kg_w"]
    gate_w = i["gate_w"]
    w357 = (i["w3"], i["w5"], i["w7"])

    dg = np.zeros((D, 9, D4 + A), np.float32)
    gr = np.zeros((C, 9, A), np.float32)
    for j in range(9):
        dy, dx = j // 3, j % 3
        dg[:, j, :D4] = de_w[:, :, dy, dx].T
        dg[:, j, D4:] = gate_w[:, C:, dy, dx].T
        gr[:, j, :] = gate_w[:, :C, dy, dx].T

    dw = np.zeros((C, K2), np.float32)
    b3 = np.zeros((C, 3), np.float32)
    for bi, (k, p, joff) in enumerate(BR):
        dw[:, joff: joff + k * k] = w357[bi][:, 0].reshape(C, k * k)
        b3[:, bi] = i[f"b{k}"]

    ebr = np.zeros((A, K2), np.float32)
    for bi, (k, p, joff) in enumerate(BR):
        ebr[bi, joff: joff + k * k] = 1.0

    sel = np.zeros((96, 32, C), np.float32)
    for g in range(3):
        for r in range(32):
            sel[32 * g + r, r, :] = 1.0

    a1 = np.zeros((C, 2, A), np.float32)
    a1[:, 0, :] = i["a1_w"][:, :C, 0, 0].T / (H * W)
    a1[:, 1, :] = i["a1_w"][:, C:, 0, 0].T / (H * W)

    return {
        "dg_w": dg, "gr_w": gr,
        "res_wt": np.ascontiguousarray(i["res_w"][:, :, 0, 0].T),
        "kg_wt": np.ascontiguousarray(kg_w[:, :, 0, 0].T),
        "dw_w": dw, "b357": b3,
        "ebr": ebr, "on3": np.ones((A, K2), np.float32),
        "one1": np.ones((1, C), np.float32),
        "zpad": np.zeros((K2P - K2, RO, W), np.float32),
        "one1f": np.ones((1, C), np.float32),
        "on3f": np.ones((A, 1), np.float32),
        "sel32": sel,
        "a1w": a1, "a2w": np.ascontiguousarray(i["a2_w"][:, :, 0, 0].T),
        "a1b": np.ascontiguousarray(i["a1_b"].reshape(A, 1)),
        "a2b": np.ascontiguousarray(i["a2_b"].reshape(A, 1)),
        "bn_g": np.ascontiguousarray(i["bn_g"].reshape(D4, 1)),
        "bn_b": np.ascontiguousarray(i["bn_b"].reshape(D4, 1)),
        "kg_b": np.ascontiguousarray(i["kg_b"].reshape(K2, 1)),
        "gate_b": np.ascontiguousarray(i["gate_b"].reshape(A, 1)),
        "res_b": np.ascontiguousarray(i["res_b"].reshape(C, 1)),
    }


def _pad_slice(x, r0, r1, c0, c1):
    """x[rows r0:r1, cols c0:c1] with zero fill outside; x is [C,H,W]."""
    ch = x.shape[0]
    outp = np.zeros((ch, r1 - r0, c1 - c0), np.float32)
    rs, re = max(r0, 0), min(r1, x.shape[1])
    cs, ce = max(c0, 0), min(c1, x.shape[2])
    if rs < re and cs < ce:
        outp[:, rs - r0: re - r0, cs - c0: ce - c0] = x[:, rs:re, cs:ce]
    return outp


def kernel(**inputs):
    inputs = {k: np.asarray(v) for k, v in inputs.items()}
    nc = _get_nc()
    common = _prep_common(inputs)

    in_maps = []
    for core in range(N_CORES):
        b, half = core // 2, core % 2
        r0 = RO * half
        m = dict(common)
        m["rgb"] = _pad_slice(inputs["rgb_feat"][b], r0 - 6, r0 + RO + 6, -6, W + 6)
        m["dep"] = _pad_slice(inputs["depth_feat"][b], r0 - 1, r0 + RO + 1, -1, W + 1)
        gm = np.zeros((C, 8), np.float32)
        gm[:, b] = 1.0
        gm[:, 4 + b] = 1.0
        m["gmask"] = gm
        hf = np.zeros((C, 2), np.float32)
        hf[:, 0] = 1.0 if half == 1 else 0.0   # keep top halo rows (inside image)?
        hf[:, 1] = 1.0 if half == 0 else 0.0   # keep bottom halo rows?
        m["half_f"] = hf
        in_maps.append(m)

    res = run_bass_kernel_spmd(nc, in_maps, core_ids=list(range(N_CORES)))

    out = np.empty((B, C, H, W), np.float32)
    for core in range(N_CORES):
        b, half = core // 2, core % 2
        out[b, :, RO * half: RO * half + RO, :] = res.results[core]["out"]
    return out



# revision 15
# speedup vs baseline: 34.6356x; 34.6356x over previous
"""Bass/Trainium2 kernel for nn_DKPF_Attention (dense_cnn).

Sharding: 8 cores = (batch b in 0..3) x (H-half in 0..1); each core computes
C=128 x 32 rows x 64 cols of the output. Cross-core traffic is one tiny
AllReduce with BatchNorm batch stats + global-average-pool partial sums.

I/O is packed into ONE bf16 tensor per core ("blob"): rgb/dep halo slices
(bf16, no column padding), all matmul weights (bf16), and a small f32
section stored as bf16 bit-pairs recovered on device via bitcast. Output is
bf16. Constant tensors (one-hot tap selectors, ones, branch masks) are
synthesized on device with memsets. This minimizes per-call host<->device
marshalling, which dominates the measured execution time.

Self-contained: shapes/weight transforms hardcoded; no external file reads.
"""

import os

import numpy as np
import ml_dtypes

import concourse.bass as bass
import concourse.bacc as bacc
import concourse.tile as tile
from concourse import mybir
from concourse.bass_utils import run_bass_kernel_spmd

F32 = mybir.dt.float32
BF16 = mybir.dt.bfloat16
AT = mybir.ActivationFunctionType
OP = mybir.AluOpType
AX = mybir.AxisListType
BF = ml_dtypes.bfloat16

B, C, D, H, W = 4, 128, 128, 64, 64
D4 = D // 4            # 32
K2 = 83                # 9 + 25 + 49
A = 3
N_CORES = 8
RO = 32                # output rows per core
NQ = 4                 # 512-px psum chunks
QR = RO // NQ          # 8 rows per chunk
EPS = 1e-5
NTOT = float(B * H * W)

# branch table: (k, pad, j-offset into the 83 kernel rows)
BR = ((3, 1, 0), (5, 2, 9), (7, 3, 34))

RGB_R = RO + 12        # 44 rows (halo 6 = dw 3 + dyn 3)
DEP_R = RO + 2         # 34 rows (halo 1)

# ---- blob layout (bf16 columns) ----
O_RGB = 0
O_DEP = O_RGB + RGB_R * W          # 2816
O_WTS = O_DEP + DEP_R * W          # 4992 — weights region start
# offsets relative to O_WTS:
R_DG = 0                           # [128, 9, 35]
R_GR = R_DG + 9 * (D4 + A)         # 315: [128, 9, 3]
R_RES = R_GR + 9 * A               # 342: [128, 128]
R_DW = R_RES + C                   # 470: [128, 83]
R_KG = R_DW + K2                   # 553: rows 0..32 = [32, 83]
R_F32 = R_KG + K2                  # 636: f32 section (58 bf16 cols = 29 f32)
NF = 29
R_SEL = R_F32 + 2 * NF             # 694: rows 0..96 = one-hot selcol [96, 32]
R_EBR = R_SEL + D4                 # 726: rows 0..3 = branch one-hot [3, 83]
WCOLS = R_EBR + K2 + 1             # 810 (even: f32 bitcast needs even pitch)
NB = O_WTS + WCOLS                 # 5802

# f32-section column indices (units of f32 columns)
FC_BNG, FC_BNB, FC_KGB, FC_GATEB, FC_RESB, FC_A1B, FC_A2B = 0, 1, 2, 3, 4, 5, 6
FC_A2W = 7            # 3 cols, rows 0..3:  a2_w.T
FC_A1W = 10           # 6 cols: [128, 2, 3] (pre-divided by H*W)
FC_B357 = 16          # 3 cols
FC_GMASK = 19         # 8 cols
FC_HALF = 27          # 2 cols


def _bcast_free(ap2, rows, cols):
    """[P,1] slice -> [P,rows,cols] free-dim broadcast AP."""
    return bass.AP(tensor=ap2.tensor, offset=ap2.offset,
                   ap=[ap2.ap[0], [0, rows], [0, cols]])


def _build():
    ABL = set(os.environ.get("KERNEL_ABLATE", "").split(","))
    nc = bacc.Bacc()

    blob = nc.declare_dram_parameter("blob", [C, NB], BF16, isOutput=False)
    out = nc.declare_dram_parameter("out", [C, RO * W], BF16, isOutput=True)

    with tile.TileContext(nc) as tc:
        with (
            nc.allow_low_precision(reason="bf16 data path is intentional"),
            tc.tile_pool(name="big", bufs=1) as big,
            tc.tile_pool(name="ev", bufs=1) as ev,
            tc.tile_pool(name="pq", bufs=1, space="PSUM") as pq,
            tc.tile_pool(name="dram", bufs=1, space="DRAM") as dram,
        ):
            # ---------------- loads + synthesized constants ----------------
            rgb_h = big.tile([C, RGB_R, W + 12], BF16)   # cols 6..70 live
            dep_h = big.tile([D, DEP_R, W + 2], BF16)    # cols 1..65 live
            wts = big.tile([C, WCOLS], BF16)
            nc.vector.memset(rgb_h[:, :, 0:6], 0.0)
            nc.vector.memset(rgb_h[:, :, W + 6: W + 12], 0.0)
            nc.vector.memset(dep_h[:, :, 0:1], 0.0)
            nc.vector.memset(dep_h[:, :, W + 1: W + 2], 0.0)
            nc.sync.dma_start(
                out=rgb_h[:, :, 6: 6 + W],
                in_=blob[:, O_RGB: O_RGB + RGB_R * W].rearrange(
                    "p (r c) -> p r c", r=RGB_R))
            nc.sync.dma_start(
                out=dep_h[:, :, 1: 1 + W],
                in_=blob[:, O_DEP: O_DEP + DEP_R * W].rearrange(
                    "p (r c) -> p r c", r=DEP_R))
            nc.sync.dma_start(out=wts, in_=blob[:, O_WTS: O_WTS + WCOLS])

            # weight views
            dg_v = wts[:, R_DG: R_DG + 9 * (D4 + A)].rearrange(
                "p (j o) -> p j o", j=9)
            gr_v = wts[:, R_GR: R_GR + 9 * A].rearrange("p (j o) -> p j o", j=9)
            res_v = wts[:, R_RES: R_RES + C]
            dw_v = wts[:, R_DW: R_DW + K2]
            kg_v = wts[:D4, R_KG: R_KG + K2]
            wf = wts[:, R_F32: R_F32 + 2 * NF].bitcast(F32)   # [128, 29] f32
            bn_g = wf[:D4, FC_BNG: FC_BNG + 1]
            bn_b = wf[:D4, FC_BNB: FC_BNB + 1]
            kg_b = wf[:K2, FC_KGB: FC_KGB + 1]
            gate_b = wf[:A, FC_GATEB: FC_GATEB + 1]
            res_b = wf[:, FC_RESB: FC_RESB + 1]
            a1b = wf[:A, FC_A1B: FC_A1B + 1]
            a2b = wf[:A, FC_A2B: FC_A2B + 1]
            a2w = wf[:A, FC_A2W: FC_A2W + A]
            a1w = wf[:, FC_A1W: FC_A1W + 6].rearrange("p (a b) -> p a b", a=2)
            b357 = wf[:, FC_B357: FC_B357 + 3]
            gmask = wf[:, FC_GMASK: FC_GMASK + 8]
            half_t = wf[:, FC_HALF: FC_HALF + 2]

            # synthesized constants
            ones_f = ev.tile([C, 1], F32)
            nc.vector.memset(ones_f, 1.0)
            ones_f3 = ev.tile([1, 4], F32)
            nc.vector.memset(ones_f3, 1.0)
            ebr = wts[:A, R_EBR: R_EBR + K2]
            # expand shipped one-hot columns [96, 32] -> [96, 32, 128] via a
            # stride-0 free-dim broadcast copy
            sel = big.tile([96, D4, C], BF16)
            selcol = wts[0:96, R_SEL: R_SEL + D4]
            selb = bass.AP(tensor=selcol.tensor, offset=selcol.offset,
                           ap=[selcol.ap[0], selcol.ap[1], [0, C]])
            nc.scalar.activation(out=sel, in_=selb, func=AT.Copy)

            # ---------------- depth-encoder conv (+ gate depth-half) --------
            e_sb = big.tile([D4, RO, W], F32)
            g_dep = big.tile([A, RO, W], F32)
            s1p = ev.tile([D4, NQ], F32)
            s2p = ev.tile([D4, NQ], F32)
            e_scr = big.tile([D4, RO, W], F32)
            for q in range(NQ):
                e_ps = pq.tile([D4 + A, QR, W], F32, tag="ps1", bufs=2)
                for j in range(9):
                    dy, dx = j // 3, j % 3
                    nc.tensor.matmul(
                        e_ps,
                        lhsT=dg_v[:, j, :],
                        rhs=dep_h[:, dy + QR * q: dy + QR * q + QR, dx: dx + W],
                        start=(j == 0), stop=(j == 8),
                    )
                qs = slice(QR * q, QR * q + QR)
                nc.vector.tensor_reduce(out=s1p[:, q: q + 1], in_=e_ps[:D4],
                                        axis=AX.XY, op=OP.add)
                nc.scalar.activation(out=e_scr[:, qs], in_=e_ps[:D4],
                                     func=AT.Square, accum_out=s2p[:, q: q + 1])
                nc.scalar.activation(out=e_sb[:, qs], in_=e_ps[:D4], func=AT.Copy)
                nc.scalar.activation(out=g_dep[:, qs], in_=e_ps[D4:], func=AT.Copy)

            # ---------------- residual 1x1 conv (early; overlaps collective) -
            res_sb = big.tile([C, RO, W], F32)
            for q in range(NQ):
                res_ps = pq.tile([C, QR, W], F32, tag="ps1", bufs=2)
                nc.tensor.matmul(res_ps, lhsT=res_v,
                                 rhs=rgb_h[:, 6 + QR * q: 6 + QR * q + QR, 6: 6 + W],
                                 start=True, stop=True)
                nc.scalar.activation(out=res_sb[:, QR * q: QR * q + QR],
                                     in_=res_ps, func=AT.Copy)

            # ---------------- GAP partials + BN stats -> AllReduce ----------
            cc_sb = ev.tile([C, 10], F32)
            nc.vector.memset(cc_sb, 0.0)
            sums2 = ev.tile([C, 2], F32)
            nc.vector.tensor_reduce(out=sums2[:, 0:1],
                                    in_=rgb_h[:, 6:6 + RO, 6:6 + W],
                                    axis=AX.XY, op=OP.add)
            nc.vector.tensor_reduce(out=sums2[:, 1:2],
                                    in_=dep_h[:, 1:1 + RO, 1:1 + W],
                                    axis=AX.XY, op=OP.add)
            s2b = bass.AP(tensor=sums2.tensor, offset=sums2.offset,
                          ap=[sums2.ap[0], sums2.ap[1], [0, 4]])
            nc.vector.tensor_tensor(
                out=cc_sb[:, 0:8].rearrange("p (a b) -> p a b", a=2),
                in0=s2b, in1=gmask.rearrange("p (a b) -> p a b", a=2),
                op=OP.mult)
            nc.vector.tensor_reduce(out=cc_sb[:D4, 8:9], in_=s1p, axis=AX.X, op=OP.add)
            nc.vector.tensor_reduce(out=cc_sb[:D4, 9:10], in_=s2p, axis=AX.X, op=OP.add)

            cc_in = dram.tile([C, 10], F32)
            cc_out = dram.tile([C, 10], F32)
            nc.sync.dma_start(out=cc_in, in_=cc_sb)
            if "cc" in ABL:
                nc.sync.dma_start(out=cc_out[:, :], in_=cc_in[:, :])
            else:
                nc.gpsimd.collective_compute(
                    "AllReduce", OP.add,
                    replica_groups=[list(range(N_CORES))],
                    ins=[cc_in[:, :]], outs=[cc_out[:, :]],
                )
            cc_r = ev.tile([C, 10], F32)
            nc.sync.dma_start(out=cc_r, in_=cc_out[:, :])

            # ---------------- depthwise convs (overlap collective) ----------
            f_ts = []
            for bi, (k, p, joff) in enumerate(BR):
                rows, cols = RO + 2 * p, W + 2 * p
                f_t = big.tile([C, rows, cols], BF16, name=f"f{k}")
                f_ts.append(f_t)
                base = 6 - 2 * p
                eng = nc.vector   # Pool engine lacks TensorScalarPtr
                bias_b = _bcast_free(b357[:, bi: bi + 1], rows, cols)
                for j in (range(1) if "dw" in ABL else range(k * k)):
                    di, dj = j // k, j % k
                    src = rgb_h[:, base + di: base + di + rows,
                                base + dj: base + dj + cols]
                    eng.scalar_tensor_tensor(
                        out=f_t, in0=src,
                        scalar=dw_v[:, joff + j: joff + j + 1],
                        in1=(bias_b if j == 0 else f_t), op0=OP.mult, op1=OP.add)
                # zero outside-image halo
                nc.gpsimd.memset(f_t[:, :, 0:p], 0.0)
                nc.gpsimd.memset(f_t[:, :, W + p: W + 2 * p], 0.0)
                nc.vector.tensor_scalar_mul(out=f_t[:, 0:p, :], in0=f_t[:, 0:p, :],
                                            scalar1=half_t[:, 0:1])
                nc.vector.tensor_scalar_mul(out=f_t[:, RO + p: RO + 2 * p, :],
                                            in0=f_t[:, RO + p: RO + 2 * p, :],
                                            scalar1=half_t[:, 1:2])

            # ---------------- gates: rgb half + softmax input ---------------
            exin = big.tile([A, RO, W], F32)
            for q in range(NQ):
                g_ps = pq.tile([A, QR, W], F32, tag="ps1", bufs=2)
                for j in range(9):
                    dy, dx = j // 3, j % 3
                    nc.tensor.matmul(
                        g_ps,
                        lhsT=gr_v[:, j, :],
                        rhs=rgb_h[:, 5 + dy + QR * q: 5 + dy + QR * q + QR,
                                  5 + dx: 5 + dx + W],
                        start=(j == 0), stop=(j == 8),
                    )
                qs = slice(QR * q, QR * q + QR)
                nc.vector.scalar_tensor_tensor(out=exin[:, qs], in0=g_ps,
                                               scalar=gate_b, in1=g_dep[:, qs],
                                               op0=OP.add, op1=OP.add)
            ex_sb = big.tile([A, RO, W], F32)
            nc.scalar.activation(out=ex_sb, in_=exin, func=AT.Exp)

            # ---------------- BN constants ----------------
            mstat = ev.tile([D4, 4], F32)
            nc.scalar.activation(out=mstat[:, 0:1], in_=cc_r[:D4, 8:9],
                                 func=AT.Copy, scale=1.0 / NTOT)
            nc.scalar.activation(out=mstat[:, 1:2], in_=cc_r[:D4, 9:10],
                                 func=AT.Copy, scale=1.0 / NTOT)
            m2 = ev.tile([D4, 1], F32)
            nc.vector.tensor_tensor(out=m2, in0=mstat[:, 0:1], in1=mstat[:, 0:1],
                                    op=OP.mult)
            nc.vector.tensor_tensor(out=mstat[:, 2:3], in0=mstat[:, 1:2], in1=m2,
                                    op=OP.subtract)
            epsb = ev.tile([D4, 1], F32)
            nc.vector.memset(epsb, EPS)
            nc.scalar.activation(out=mstat[:, 2:3], in_=mstat[:, 2:3],
                                 func=AT.Sqrt, bias=epsb)
            nc.vector.reciprocal(out=mstat[:, 3:4], in_=mstat[:, 2:3])
            bnsc = ev.tile([D4, 1], F32)
            bnsh = ev.tile([D4, 1], F32)
            nc.vector.tensor_tensor(out=bnsc, in0=mstat[:, 3:4], in1=bn_g, op=OP.mult)
            msc = ev.tile([D4, 1], F32)
            nc.vector.tensor_tensor(out=msc, in0=mstat[:, 0:1], in1=bnsc, op=OP.mult)
            nc.vector.tensor_tensor(out=bnsh, in0=bn_b, in1=msc, op=OP.subtract)

            de_bf = big.tile([D4, RO, W], BF16)
            nc.scalar.activation(out=de_bf, in_=e_sb, func=AT.Relu,
                                 bias=bnsh, scale=bnsc)

            # ---------------- attention (tiny) ----------------
            apk = ev.tile([C, 2, 4], F32)
            nc.vector.tensor_tensor(out=apk,
                                    in0=cc_r[:, 0:8].rearrange("p (a b) -> p a b", a=2),
                                    in1=gmask.rearrange("p (a b) -> p a b", a=2),
                                    op=OP.mult)
            a_sel = ev.tile([C, 2], F32)
            nc.vector.tensor_reduce(out=a_sel, in_=apk, axis=AX.X, op=OP.add)
            a1_ps = pq.tile([A, QR, W], F32, tag="ps1", bufs=2)
            nc.tensor.matmul(a1_ps[:, 0, 0:1], lhsT=a1w[:, 0, :], rhs=a_sel[:, 0:1],
                             start=True, stop=False)
            nc.tensor.matmul(a1_ps[:, 0, 0:1], lhsT=a1w[:, 1, :], rhs=a_sel[:, 1:2],
                             start=False, stop=True)
            h_r = ev.tile([A, 1], F32)
            nc.scalar.activation(out=h_r, in_=a1_ps[:, 0, 0:1], func=AT.Relu, bias=a1b)
            a2_ps = pq.tile([A, QR, W], F32, tag="ps1", bufs=2)
            nc.tensor.matmul(a2_ps[:, 0, 0:1], lhsT=a2w, rhs=h_r, start=True, stop=True)
            exq = ev.tile([A, 1], F32)
            nc.scalar.activation(out=exq, in_=a2_ps[:, 0, 0:1], func=AT.Exp, bias=a2b)
            sq_ps = pq.tile([1, QR, W], F32, tag="ps1", bufs=2)
            nc.tensor.matmul(sq_ps[:, 0, 0:1], lhsT=exq, rhs=ones_f[:A], start=True,
                             stop=True)
            rs = ev.tile([1, 1], F32)
            nc.vector.reciprocal(out=rs, in_=sq_ps[:, 0, 0:1])
            rb_ps = pq.tile([A, QR, W], F32, tag="ps1", bufs=2)
            nc.tensor.matmul(rb_ps[:, 0, 0:1], lhsT=ones_f3[0:1, 0:A], rhs=rs,
                             start=True, stop=True)
            att = ev.tile([A, 1], F32)
            nc.vector.tensor_tensor(out=att, in0=exq, in1=rb_ps[:, 0, 0:1], op=OP.mult)

            # ---------------- kern assembly ----------------
            # kern[j] = (kw[j] + kg_b[j]) * exa[br(j)] / sum_b ex[b]
            exa_sb = big.tile([A, RO, W], F32)
            nc.vector.tensor_scalar_mul(out=exa_sb, in0=ex_sb, scalar1=att)
            rsum_sb = big.tile([1, RO, W], F32)
            wfac_bf = big.tile([A, RO, W], BF16)
            exab_sb = big.tile([K2, RO, W], BF16)
            kern_bf = big.tile([96, RO, W], BF16)
            nc.vector.memset(kern_bf, 0.0)
            for q in range(NQ):
                qs = slice(QR * q, QR * q + QR)
                sum_ps = pq.tile([1, QR, W], F32, tag="ps1", bufs=2)
                nc.tensor.matmul(sum_ps, lhsT=ones_f[:A], rhs=ex_sb[:, qs],
                                 start=True, stop=True)
                nc.vector.reciprocal(out=rsum_sb[:, qs], in_=sum_ps)
                rb3_ps = pq.tile([A, QR, W], F32, tag="ps1", bufs=2)
                nc.tensor.matmul(rb3_ps, lhsT=ones_f3[0:1, 0:A], rhs=rsum_sb[:, qs],
                                 start=True, stop=True)
                nc.vector.tensor_tensor(out=wfac_bf[:, qs], in0=exa_sb[:, qs],
                                        in1=rb3_ps, op=OP.mult)
                wfull_ps = pq.tile([K2, QR, W], F32, tag="ps1", bufs=2)
                nc.tensor.matmul(wfull_ps, lhsT=ebr, rhs=wfac_bf[:, qs],
                                 start=True, stop=True)
                nc.scalar.activation(out=exab_sb[:, qs], in_=wfull_ps, func=AT.Copy)
                kw_ps = pq.tile([K2, QR, W], F32, tag="ps1", bufs=2)
                nc.tensor.matmul(kw_ps, lhsT=kg_v, rhs=de_bf[:, qs],
                                 start=True, stop=True)
                nc.vector.scalar_tensor_tensor(out=kern_bf[:K2, qs], in0=kw_ps,
                                               scalar=kg_b, in1=exab_sb[:, qs],
                                               op0=OP.add, op1=OP.mult)

            # ---------------- dynamic conv ----------------
            HC = 2                 # kb half-chunks of 16 rows (2 psum banks)
            HR = RO // HC          # 16
            acc_v = big.tile([C, RO, W], BF16)
            acc_g = big.tile([C, RO, W], BF16)
            started = {}
            ti = 0
            for bi, (k, p, joff) in enumerate(BR):
                f_t = f_ts[bi]
                for j in (range(0) if "dyn" in ABL else range(k * k)):
                    di, dj = j // k, j % k
                    jj = joff + j
                    g, r = (jj // 32) * 32, jj % 32
                    eng, acc = ((nc.vector, acc_v) if ti % 2 == 0
                                else (nc.gpsimd, acc_g))
                    ti += 1
                    for h in range(HC):
                        kb_ps = pq.tile([C, HR, W], F32, tag="kb", bufs=3)
                        for s in range(2):
                            nc.tensor.matmul(
                                kb_ps[:, QR * s: QR * s + QR, :],
                                lhsT=sel[g: g + D4, r, :],
                                rhs=kern_bf[g: g + D4,
                                            HR * h + QR * s: HR * h + QR * s + QR, :],
                                start=True, stop=True)
                        kb_sb = ev.tile([C, HR, W], BF16, tag="kbs", bufs=4)
                        nc.scalar.activation(out=kb_sb, in_=kb_ps, func=AT.Copy)
                        fsl = f_t[:, di + HR * h: di + HR * h + HR, dj: dj + W]
                        key = (ti % 2, h)
                        asl = acc[:, HR * h: HR * h + HR, :]
                        if key not in started:
                            started[key] = True
                            eng.tensor_tensor(out=asl, in0=fsl, in1=kb_sb, op=OP.mult)
                        else:
                            tmp = ev.tile([C, HR, W], BF16, tag="tmp", bufs=4)
                            eng.tensor_tensor(out=tmp, in0=fsl, in1=kb_sb, op=OP.mult)
                            eng.tensor_tensor(out=asl, in0=asl, in1=tmp, op=OP.add)

            # ---------------- combine + store ----------------
            out_sb = big.tile([C, RO, W], BF16)
            if "dyn" in ABL:
                nc.vector.scalar_tensor_tensor(out=out_sb, in0=res_sb, scalar=res_b,
                                               in1=res_sb, op0=OP.add, op1=OP.bypass)
            else:
                t1 = big.tile([C, RO, W], BF16)
                nc.vector.tensor_tensor(out=t1, in0=acc_v, in1=acc_g, op=OP.add)
                nc.vector.scalar_tensor_tensor(out=out_sb, in0=res_sb, scalar=res_b,
                                               in1=t1, op0=OP.add, op1=OP.add)
            nc.sync.dma_start(out=out[...],
                              in_=out_sb.rearrange("p r c -> p (r c)"))

    nc.finalize()
    return nc


_NC = None


def _get_nc():
    global _NC
    if _NC is None:
        _NC = _build()
    return _NC


def _prep_wts(i):
    """Shared weights region [128, WCOLS] as bf16 (f32 section bit-packed)."""
    de_w, kg_w, gate_w = i["de_w"], i["kg_w"], i["gate_w"]
    w357 = (i["w3"], i["w5"], i["w7"])

    wts = np.zeros((C, WCOLS), np.uint16)

    def put_bf(col, arr2d):
        a = np.asarray(arr2d, np.float32).astype(BF).view(np.uint16)
        wts[: a.shape[0], col: col + a.shape[1]] = a

    dg = np.zeros((D, 9, D4 + A), np.float32)
    gr = np.zeros((C, 9, A), np.float32)
    for j in range(9):
        dy, dx = j // 3, j % 3
        dg[:, j, :D4] = de_w[:, :, dy, dx].T
        dg[:, j, D4:] = gate_w[:, C:, dy, dx].T
        gr[:, j, :] = gate_w[:, :C, dy, dx].T
    put_bf(R_DG, dg.reshape(D, -1))
    put_bf(R_GR, gr.reshape(C, -1))
    put_bf(R_RES, i["res_w"][:, :, 0, 0].T)

    dw = np.zeros((C, K2), np.float32)
    b3 = np.zeros((C, 3), np.float32)
    for bi, (k, p, joff) in enumerate(BR):
        dw[:, joff: joff + k * k] = w357[bi][:, 0].reshape(C, k * k)
        b3[:, bi] = i[f"b{k}"]
    put_bf(R_DW, dw)
    put_bf(R_KG, kg_w[:, :, 0, 0].T)

    f32 = np.zeros((C, NF), np.float32)
    f32[:D4, FC_BNG] = i["bn_g"]
    f32[:D4, FC_BNB] = i["bn_b"]
    f32[:K2, FC_KGB] = i["kg_b"]
    f32[:A, FC_GATEB] = i["gate_b"]
    f32[:, FC_RESB] = i["res_b"]
    f32[:A, FC_A1B] = i["a1_b"]
    f32[:A, FC_A2B] = i["a2_b"]
    f32[:A, FC_A2W: FC_A2W + A] = i["a2_w"][:, :, 0, 0].T
    a1 = np.zeros((C, 2, A), np.float32)
    a1[:, 0, :] = i["a1_w"][:, :C, 0, 0].T / (H * W)
    a1[:, 1, :] = i["a1_w"][:, C:, 0, 0].T / (H * W)
    f32[:, FC_A1W: FC_A1W + 6] = a1.reshape(C, 6)
    f32[:, FC_B357: FC_B357 + 3] = b3
    wts[:, R_F32: R_F32 + 2 * NF] = f32.view(np.uint16)

    selcol = np.zeros((96, D4), np.float32)
    for p in range(96):
        selcol[p, p % D4] = 1.0
    wts[:96, R_SEL: R_SEL + D4] = selcol.astype(BF).view(np.uint16)

    ebr = np.zeros((A, K2), np.float32)
    for bi, (k, p, joff) in enumerate(BR):
        ebr[bi, joff: joff + k * k] = 1.0
    wts[:A, R_EBR: R_EBR + K2] = ebr.astype(BF).view(np.uint16)
    return wts


def _pad_slice_bf(x, r0, r1):
    """x[:, r0:r1, :] with zero fill outside rows; x is [C,H,W] f32 -> bf16."""
    ch = x.shape[0]
    outp = np.zeros((ch, r1 - r0, x.shape[2]), BF)
    rs, re = max(r0, 0), min(r1, x.shape[1])
    if rs < re:
        outp[:, rs - r0: re - r0, :] = x[:, rs:re, :].astype(BF)
    return outp


def _build_in_maps(inputs):
    wts_u16 = _prep_wts(inputs)
    in_maps = []
    for core in range(N_CORES):
        b, half = core // 2, core % 2
        r0 = RO * half
        blob = np.zeros((C, NB), np.uint16)
        blob[:, O_RGB: O_RGB + RGB_R * W] = _pad_slice_bf(
            inputs["rgb_feat"][b], r0 - 6, r0 + RO + 6).view(np.uint16).reshape(C, -1)
        blob[:, O_DEP: O_DEP + DEP_R * W] = _pad_slice_bf(
            inputs["depth_feat"][b], r0 - 1, r0 + RO + 1).view(np.uint16).reshape(C, -1)
        blob[:, O_WTS:] = wts_u16
        # per-core f32 extras: gmask + halo-row keep masks
        gm = np.zeros((C, 8), np.float32)
        gm[:, b] = 1.0
        gm[:, 4 + b] = 1.0
        blob[:, O_WTS + R_F32 + 2 * FC_GMASK: O_WTS + R_F32 + 2 * (FC_GMASK + 8)] = \
            gm.view(np.uint16)
        hf = np.zeros((C, 2), np.float32)
        hf[:, 0] = 1.0 if half == 1 else 0.0
        hf[:, 1] = 1.0 if half == 0 else 0.0
        blob[:, O_WTS + R_F32 + 2 * FC_HALF: O_WTS + R_F32 + 2 * (FC_HALF + 2)] = \
            hf.view(np.uint16)
        in_maps.append({"blob": blob.view(BF)})
    return in_maps


def kernel(**inputs):
    inputs = {k: np.asarray(v) for k, v in inputs.items()}
    nc = _get_nc()
    in_maps = _build_in_maps(inputs)
    res = run_bass_kernel_spmd(nc, in_maps, core_ids=list(range(N_CORES)))

    out = np.empty((B, C, H, W), np.float32)
    for core in range(N_CORES):
        b, half = core // 2, core % 2
        o = np.asarray(res.results[core]["out"]).astype(np.float32)
        out[b, :, RO * half: RO * half + RO, :] = o.reshape(C, RO, W)
    return out
